# revision 28
# baseline (speedup 1.0000x reference)
"""AVWGCN Trainium2 kernel: adaptive-adjacency Chebyshev GCN.

Math (per core, batch-sharded over B: 8 batches/core):
  A = relu(E @ E^T) (symmetric), M = exp(A), r = rowsum(M), S = diag(1/r) M
  X2[m,(b,c)] = x[b,m,c]
  x1 = diag(1/r) (M @ X2)
  x2 = 2 diag(1/r) (M @ x1) - X2
  out[b,n,o] = sum_d E[n,d] * ( sum_{k,i} xg_k[n,(b,i)] Wp[d,k,i,o] + bp[d,o] )
Key tricks:
  - M symmetric -> its [n-part, m-free] tiles serve directly as matmul lhsT.
  - 1/r scalings are per-output-row -> per-partition DVE scalars.
  - gconv via Z-form: Z[bn,(o,d)] = XGT.T @ Wp_flat; bias folded in as a
    ones-row appended to the k2 K-chunk; epilogue contracts d with
    broadcast-replicated E on DVE.
  - x_g transposed to [ki-part, (b,m)-free] via PE transposes (k=0 slice comes
    pre-transposed from the host).
"""

from contextlib import ExitStack

import numpy as np

import concourse.bass as bass
import concourse.mybir as mybir
import concourse.tile as tile
from concourse.bass_utils import run_bass_kernel_spmd
from concourse.masks import make_identity

B, N, C, CHEB_K, EMBED = 64, 2048, 64, 3, 16
NCORES = 8
BC = B // NCORES            # batches per core
F = BC * C                  # 512: free width of X2 [m, (b,c)]
NT = N // 128               # 16 n-chunks
FP32 = mybir.dt.float32
BF16 = mybir.dt.bfloat16
MM_DT = BF16                # dtype of big-matmul operands
DO = C * EMBED              # 1024, Z free width, (o, d) ordered


_WAIT_CAP = {"InstDMACopy": 1}
_WAIT_SAFE = {"InstEventSemaphore", "InstCall",
              "InstUnconditionalBranch", "InstISA", "InstRegisterMove"}


def _split_excess_waits(nc):
    """Walrus rejects compute instructions carrying more sync waits than the
    ISA struct can encode. Hoist excess waits onto an inserted same-engine
    Drain immediately before the instruction (semantically identical)."""
    SyncInfo = None
    n_fix = 0
    for f in nc.m.functions:
        for blk in f.blocks:
            out_insts = []
            for inst in blk.instructions:
                tn = type(inst).__name__
                si = inst.sync_info
                w = list(si.on_wait) if (si is not None and si.on_wait) else []
                cap = _WAIT_CAP.get(tn, 1)
                if tn not in _WAIT_SAFE and len(w) > cap:
                    if SyncInfo is None:
                        SyncInfo = type(si)
                    for wx in w:
                        d = mybir.InstDrain(name=f"I-wsplit{nc.next_id()}",
                                            ins=[], outs=[])
                        d.engine = inst.engine
                        d.sync_info = SyncInfo(on_wait=[wx], on_update=[])
                        out_insts.append(d)
                    si.on_wait = []
                    n_fix += 1
                out_insts.append(inst)
            blk.instructions[:] = out_insts
    return n_fix


def build_nc():
    nc = bass.Bass()
    x2d = nc.dram_tensor("x2d", [N, F], FP32, kind="ExternalInput").ap()
    xt = nc.dram_tensor("xt", [C, BC, N], FP32, kind="ExternalInput").ap()
    et = nc.dram_tensor("et", [EMBED, N], FP32, kind="ExternalInput").ap()
    etlo_d = nc.dram_tensor("etlo", [EMBED, N], FP32, kind="ExternalInput").ap()
    en = nc.dram_tensor("en", [N, EMBED], FP32, kind="ExternalInput").ap()
    wpf1 = nc.dram_tensor("wpf1", [128, DO], FP32, kind="ExternalInput").ap()
    wpf2 = nc.dram_tensor("wpf2", [65, DO], FP32, kind="ExternalInput").ap()
    out = nc.dram_tensor("out", [BC, N, C], FP32, kind="ExternalOutput").ap()

    with tile.TileContext(nc) as tc:
        with ExitStack() as ctx:
            kernel_body(ctx, tc, out, x2d, xt, et, etlo_d, en, wpf1, wpf2)
    _split_excess_waits(nc)
    return nc


def kernel_body(ctx, tc, out, x2d, xt, et, etlo_d, en, wpf1, wpf2):
    nc = tc.nc

    singles = ctx.enter_context(tc.tile_pool(name="singles", bufs=1))
    erep_pool = ctx.enter_context(tc.tile_pool(name="erep", bufs=2))
    tmp_pool = ctx.enter_context(tc.tile_pool(name="tmp", bufs=8))
    zsb_pool = ctx.enter_context(tc.tile_pool(name="zsb", bufs=3))
    ps_mm = ctx.enter_context(tc.tile_pool(name="ps_mm", bufs=2, space="PSUM"))
    ps_t = ctx.enter_context(tc.tile_pool(name="ps_t", bufs=2, space="PSUM"))
    ps_z = ctx.enter_context(tc.tile_pool(name="ps_z", bufs=2, space="PSUM"))

    # ---- constants ----
    ident = singles.tile([128, 128], MM_DT, tag="ident")
    make_identity(nc, ident)
    # split-precision E^T (host-split): A = EhiEhi^T + EhiElo^T + EloEhi^T
    ethi = singles.tile([EMBED, N], MM_DT, tag="ethi")
    etlo = singles.tile([EMBED, N], MM_DT, tag="etlo")
    nc.gpsimd.dma_start(out=ethi, in_=et)
    nc.gpsimd.dma_start(out=etlo, in_=etlo_d)
    w1_sb = singles.tile([128, DO], MM_DT, tag="w1")
    nc.gpsimd.dma_start(out=w1_sb, in_=wpf1)
    w2_sb = singles.tile([65, DO], MM_DT, tag="w2")
    nc.gpsimd.dma_start(out=w2_sb, in_=wpf2)

    # XGT tiles: k0+k1 interleaved on partitions, k2 (+ones row) separate
    xgt01 = singles.tile([128, BC * N], MM_DT, tag="xgt01")      # 4 MiB
    xgt2 = singles.tile([65, BC * N], MM_DT, tag="xgt2")         # ~2 MiB
    # k=0 slice comes pre-transposed from host; cast to bf16 during DMA
    nc.gpsimd.dma_start(out=xgt01[0:C, :], in_=xt.rearrange("c b n -> c (b n)"))
    nc.vector.memset(xgt2[64:65, :], 1.0)                        # bias ones-row

    rsum4 = singles.tile([128, NT, 4], FP32, tag="rsum4")
    rinv = singles.tile([128, NT], FP32, tag="rinv")
    rinv2 = singles.tile([128, NT], FP32, tag="rinv2")

    # ---- stage 1: M = exp(relu(E E^T)) as bf16 tiles + row sums ----
    m_sb = [singles.tile([128, N], MM_DT, name=f"m{j}", tag=f"m{j}") for j in range(NT)]
    for j in range(NT):
        for q in range(4):
            pa = ps_z.tile([128, DO], FP32, name="pa", tag="pz")[:, 0:512]
            jsl = slice(j * 128, (j + 1) * 128)
            qsl = slice(q * 512, (q + 1) * 512)
            nc.tensor.matmul(pa, lhsT=ethi[:, jsl], rhs=ethi[:, qsl],
                             start=True, stop=False)
            nc.tensor.matmul(pa, lhsT=ethi[:, jsl], rhs=etlo[:, qsl],
                             start=False, stop=False)
            nc.tensor.matmul(pa, lhsT=etlo[:, jsl], rhs=ethi[:, qsl],
                             start=False, stop=True)
            dst = m_sb[j][:, q * 512:(q + 1) * 512]
            # exp(relu(a)) == max(exp(a), 1); row-sum accumulated in the max op
            nc.scalar.activation(out=dst, in_=pa,
                                 func=mybir.ActivationFunctionType.Exp)
            nc.vector.tensor_scalar(out=dst, in0=dst, scalar1=1.0, scalar2=None,
                                    op0=mybir.AluOpType.max,
                                    op1=mybir.AluOpType.add,
                                    accum_out=rsum4[:, j, q:q + 1])
    rsum = singles.tile([128, NT], FP32, tag="rsum")
    rneg2 = singles.tile([128, NT], FP32, tag="rneg2")
    nc.vector.reduce_sum(out=rsum, in_=rsum4, axis=mybir.AxisListType.X)
    nc.vector.reciprocal(out=rinv, in_=rsum)
    nc.vector.tensor_scalar_mul(out=rinv2, in0=rinv, scalar1=2.0)
    nc.vector.tensor_scalar_mul(out=rneg2, in0=rsum, scalar1=-0.5)
    # ACT-written copies so ACT consumers don't need a DVE wait
    rinv_act = singles.tile([128, NT], FP32, tag="rinv_act")
    rinv2_act = singles.tile([128, NT], FP32, tag="rinv2_act")
    nc.scalar.copy(out=rinv_act, in_=rinv)
    nc.scalar.copy(out=rinv2_act, in_=rinv2)
    # diag(-r/2) tiles: fold the X2 subtraction into the stage-4 matmul
    dneg = [singles.tile([128, 128], MM_DT, name=f"dneg{j}", tag=f"dneg{j}")
            for j in range(NT)]
    for j in range(NT):
        nc.vector.tensor_scalar_mul(out=dneg[j], in0=ident,
                                    scalar1=rneg2[:, j:j + 1])

    # ---- stage 2: load X2 ----
    x2_sb = [singles.tile([128, F], MM_DT, name=f"x2_{a}", tag=f"x2{a}") for a in range(NT)]
    for a in range(NT):
        nc.gpsimd.dma_start(out=x2_sb[a], in_=x2d[a * 128:(a + 1) * 128, :])

    # ---- stage 3: x1 = diag(1/r) M X2 ----
    x1_sb = [singles.tile([128, F], MM_DT, name=f"x1_{j}", tag=f"x1{j}") for j in range(NT)]
    for j in range(NT):
        pm = ps_mm.tile([128, F], FP32, tag="pm")
        for a in range(NT):
            nc.tensor.matmul(pm, lhsT=m_sb[a][:, j * 128:(j + 1) * 128],
                             rhs=x2_sb[a], start=(a == 0), stop=(a == NT - 1))
        nc.scalar.activation(out=x1_sb[j], in_=pm,
                             func=mybir.ActivationFunctionType.Copy,
                             scale=rinv_act[:, j:j + 1])

    # ---- stage 4: x2 = 2 diag(1/r) M x1 - X2 ----
    x2o_sb = [singles.tile([128, F], MM_DT, name=f"x2o_{j}", tag=f"x2o{j}") for j in range(NT)]
    for j in range(NT):
        pm = ps_mm.tile([128, F], FP32, tag="pm")
        for a in range(NT):
            nc.tensor.matmul(pm, lhsT=m_sb[a][:, j * 128:(j + 1) * 128],
                             rhs=x1_sb[a], start=(a == 0), stop=False)
        # psum += diag(-r/2) @ X2[j]  => psum = M@x1 - (r/2) X2[j]
        nc.tensor.matmul(pm, lhsT=dneg[j], rhs=x2_sb[j], start=False, stop=True)
        # x2 = (2/r) * psum
        nc.scalar.activation(out=x2o_sb[j], in_=pm,
                             func=mybir.ActivationFunctionType.Copy,
                             scale=rinv2_act[:, j:j + 1])

    # ---- stage 5: PE-transpose x1 -> xgt01[64:128], x2 -> xgt2[0:64] ----
    for j in range(NT):
        for b in range(BC):
            col = b * N + j * 128
            tp = ps_t.tile([128, 128], MM_DT, tag="tp")
            nc.tensor.transpose(tp[64:128, :], x1_sb[j][:, b * C:(b + 1) * C],
                                ident, tile_position=(0, 64))
            nc.scalar.copy(out=xgt01[64:128, col:col + 128], in_=tp[64:128, :])
            tp2 = ps_t.tile([128, 128], MM_DT, tag="tp")
            nc.tensor.transpose(tp2[0:64, :], x2o_sb[j][:, b * C:(b + 1) * C],
                                ident, tile_position=(0, 0))
            nc.scalar.copy(out=xgt2[0:64, col:col + 128], in_=tp2[0:64, :])

    # ---- stage 6: Z matmul + epilogue d-contraction ----
    for jm in range(NT):
        erep = erep_pool.tile([128, C, EMBED], MM_DT, tag="erep")
        src = en[jm * 128:(jm + 1) * 128, :]
        bc_ap = bass.AP(tensor=src.tensor, offset=src.offset,
                        ap=[src.ap[0], [0, C], src.ap[1]])
        nc.gpsimd.dma_start(out=erep, in_=bc_ap)
        for b in range(BC):
            col = b * N + jm * 128
            pz = ps_z.tile([128, DO], FP32, tag="pz")
            for h in range(2):
                sl = slice(h * 512, (h + 1) * 512)
                nc.tensor.matmul(pz[:, sl], lhsT=xgt01[:, col:col + 128],
                                 rhs=w1_sb[:, sl], start=True, stop=False)
                nc.tensor.matmul(pz[:, sl], lhsT=xgt2[:, col:col + 128],
                                 rhs=w2_sb[:, sl], start=False, stop=True)
            zsb = zsb_pool.tile([128, C, EMBED], MM_DT, tag="zsb")
            nc.scalar.copy(out=zsb, in_=pz)       # PSUM -> SBUF, cast bf16
            nc.vector.tensor_mul(zsb, zsb, erep)  # in-place *E (bf16 2x)
            # tree-reduce over d (innermost, 16): 3 halvings + final fp32 add
            for hw_ in (8, 4, 2):
                nc.vector.tensor_add(zsb[:, :, 0:hw_], zsb[:, :, 0:hw_],
                                     zsb[:, :, hw_:2 * hw_])
            tmp = tmp_pool.tile([128, C], FP32, tag="tmp")
            nc.vector.tensor_add(tmp, zsb[:, :, 0], zsb[:, :, 1])
            nc.sync.dma_start(out=out[b, jm * 128:(jm + 1) * 128, :], in_=tmp)


_NC_CACHE = None


def kernel(x, node_embedding, weights_pool, bias_pool):
    global _NC_CACHE
    if _NC_CACHE is None:
        _NC_CACHE = build_nc()
    nc = _NC_CACHE

    x = np.asarray(x, dtype=np.float32)
    E = np.asarray(node_embedding, dtype=np.float32)
    Wp = np.asarray(weights_pool, dtype=np.float32)
    bp = np.asarray(bias_pool, dtype=np.float32)

    import ml_dtypes
    etf = np.ascontiguousarray(E.T)
    eth = etf.astype(ml_dtypes.bfloat16).astype(np.float32)
    et = eth
    etlo = np.ascontiguousarray(etf - eth)
    # wpf[(k,i), (o,d)] = Wp[d,k,i,o]
    wpf = np.ascontiguousarray(Wp.transpose(1, 2, 3, 0).reshape(CHEB_K * C, DO))
    wpf1 = np.ascontiguousarray(wpf[0:128])
    # last row of wpf2 = bias pool flattened (o,d) to ride the ones-row
    bprow = np.ascontiguousarray(bp.T.reshape(1, DO))
    wpf2 = np.ascontiguousarray(np.concatenate([wpf[128:192], bprow], axis=0))

    in_maps = []
    for c in range(NCORES):
        xc = x[BC * c:BC * (c + 1)]
        in_maps.append({
            "x2d": np.ascontiguousarray(xc.transpose(1, 0, 2).reshape(N, F)),
            "xt": np.ascontiguousarray(xc.transpose(2, 0, 1)),
            "et": et, "etlo": etlo, "en": E, "wpf1": wpf1, "wpf2": wpf2,
        })
    res = run_bass_kernel_spmd(nc, in_maps, list(range(NCORES)))
    return np.concatenate([res.results[c]["out"] for c in range(NCORES)], axis=0)


if __name__ == "__main__":
    rng = np.random.default_rng(0)
    inputs = {
        "x": rng.standard_normal((B, N, C), dtype=np.float32),
        "node_embedding": rng.standard_normal((N, EMBED), dtype=np.float32),
        "weights_pool": (rng.standard_normal((EMBED, CHEB_K, C, C), dtype=np.float32) * 0.1),
        "bias_pool": (rng.standard_normal((EMBED, C), dtype=np.float32) * 0.1),
    }
    got = kernel(**inputs)
    print("out", got.shape, got.dtype, np.abs(got).max())


# revision 29
# speedup vs baseline: 1.2152x; 1.2152x over previous
"""AVWGCN Trainium2 kernel: adaptive-adjacency Chebyshev GCN.

Math (per core, batch-sharded over B: 8 batches/core):
  A = relu(E @ E^T) (symmetric), M = exp(A), r = rowsum(M), S = diag(1/r) M
  X2[m,(b,c)] = x[b,m,c]
  x1 = diag(1/r) (M @ X2)
  x2 = 2 diag(1/r) (M @ x1) - X2
  out[b,n,o] = sum_d E[n,d] * ( sum_{k,i} xg_k[n,(b,i)] Wp[d,k,i,o] + bp[d,o] )
Key tricks:
  - M symmetric -> its [n-part, m-free] tiles serve directly as matmul lhsT.
  - 1/r scalings are per-output-row -> per-partition DVE scalars.
  - gconv via Z-form: Z[bn,(o,d)] = XGT.T @ Wp_flat; bias folded in as a
    ones-row appended to the k2 K-chunk; epilogue contracts d with
    broadcast-replicated E on DVE.
  - x_g transposed to [ki-part, (b,m)-free] via PE transposes (k=0 slice comes
    pre-transposed from the host).
"""

from contextlib import ExitStack

import numpy as np

import concourse.bass as bass
import concourse.mybir as mybir
import concourse.tile as tile
from concourse.bass_utils import run_bass_kernel_spmd
from concourse.masks import make_identity

B, N, C, CHEB_K, EMBED = 64, 2048, 64, 3, 16
NCORES = 8
BC = B // NCORES            # batches per core
F = BC * C                  # 512: free width of X2 [m, (b,c)]
NT = N // 128               # 16 n-chunks
FP32 = mybir.dt.float32
BF16 = mybir.dt.bfloat16
MM_DT = BF16                # dtype of big-matmul operands
DO = C * EMBED              # 1024, Z free width, (o, d) ordered


_WAIT_CAP = {"InstDMACopy": 1}
_WAIT_SAFE = {"InstEventSemaphore", "InstCall",
              "InstUnconditionalBranch", "InstISA", "InstRegisterMove"}


def _split_excess_waits(nc):
    """Walrus rejects compute instructions carrying more sync waits than the
    ISA struct can encode. Hoist excess waits onto an inserted same-engine
    Drain immediately before the instruction (semantically identical)."""
    SyncInfo = None
    n_fix = 0
    for f in nc.m.functions:
        for blk in f.blocks:
            out_insts = []
            for inst in blk.instructions:
                tn = type(inst).__name__
                si = inst.sync_info
                w = list(si.on_wait) if (si is not None and si.on_wait) else []
                cap = _WAIT_CAP.get(tn, 1)
                if tn not in _WAIT_SAFE and len(w) > cap:
                    if SyncInfo is None:
                        SyncInfo = type(si)
                    for wx in w:
                        d = mybir.InstDrain(name=f"I-wsplit{nc.next_id()}",
                                            ins=[], outs=[])
                        d.engine = inst.engine
                        d.sync_info = SyncInfo(on_wait=[wx], on_update=[])
                        out_insts.append(d)
                    si.on_wait = []
                    n_fix += 1
                out_insts.append(inst)
            blk.instructions[:] = out_insts
    return n_fix


def build_nc():
    nc = bass.Bass()
    x2d = nc.dram_tensor("x2d", [N, F], FP32, kind="ExternalInput").ap()
    xt = nc.dram_tensor("xt", [C, BC, N], FP32, kind="ExternalInput").ap()
    et = nc.dram_tensor("et", [EMBED, N], FP32, kind="ExternalInput").ap()
    etlo_d = nc.dram_tensor("etlo", [EMBED, N], FP32, kind="ExternalInput").ap()
    en = nc.dram_tensor("en", [N, EMBED], FP32, kind="ExternalInput").ap()
    wpf1 = nc.dram_tensor("wpf1", [128, DO], FP32, kind="ExternalInput").ap()
    wpf2 = nc.dram_tensor("wpf2", [65, DO], FP32, kind="ExternalInput").ap()
    out = nc.dram_tensor("out", [BC, N, C], FP32, kind="ExternalOutput").ap()

    with tile.TileContext(nc) as tc:
        with ExitStack() as ctx:
            kernel_body(ctx, tc, out, x2d, xt, et, etlo_d, en, wpf1, wpf2)
    _split_excess_waits(nc)
    return nc


def kernel_body(ctx, tc, out, x2d, xt, et, etlo_d, en, wpf1, wpf2):
    nc = tc.nc

    singles = ctx.enter_context(tc.tile_pool(name="singles", bufs=1))
    erep_pool = ctx.enter_context(tc.tile_pool(name="erep", bufs=2))
    tmp_pool = ctx.enter_context(tc.tile_pool(name="tmp", bufs=8))
    zsb_pool = ctx.enter_context(tc.tile_pool(name="zsb", bufs=3))
    ps_mm = ctx.enter_context(tc.tile_pool(name="ps_mm", bufs=2, space="PSUM"))
    ps_t = ctx.enter_context(tc.tile_pool(name="ps_t", bufs=2, space="PSUM"))
    ps_z = ctx.enter_context(tc.tile_pool(name="ps_z", bufs=2, space="PSUM"))

    # ---- constants ----
    ident = singles.tile([128, 128], MM_DT, tag="ident")
    make_identity(nc, ident)
    # split-precision E^T (host-split): A = EhiEhi^T + EhiElo^T + EloEhi^T
    ethi = singles.tile([EMBED, N], MM_DT, tag="ethi")
    etlo = singles.tile([EMBED, N], MM_DT, tag="etlo")
    nc.gpsimd.dma_start(out=ethi, in_=et)
    nc.gpsimd.dma_start(out=etlo, in_=etlo_d)
    w1_sb = singles.tile([128, DO], MM_DT, tag="w1")
    nc.gpsimd.dma_start(out=w1_sb, in_=wpf1)
    w2_sb = singles.tile([65, DO], MM_DT, tag="w2")
    nc.gpsimd.dma_start(out=w2_sb, in_=wpf2)

    # XGT tiles: k0+k1 interleaved on partitions, k2 (+ones row) separate
    xgt01 = singles.tile([128, BC * N], MM_DT, tag="xgt01")      # 4 MiB
    xgt2 = singles.tile([65, BC * N], MM_DT, tag="xgt2")         # ~2 MiB
    # k=0 slice comes pre-transposed from host; cast to bf16 during DMA
    nc.gpsimd.dma_start(out=xgt01[0:C, :], in_=xt.rearrange("c b n -> c (b n)"))
    nc.vector.memset(xgt2[64:65, :], 1.0)                        # bias ones-row

    rsum4 = singles.tile([128, NT, 4], FP32, tag="rsum4")
    rinv = singles.tile([128, NT], FP32, tag="rinv")
    rinv2 = singles.tile([128, NT], FP32, tag="rinv2")

    # ---- stage 1: M = exp(relu(E E^T)) as bf16 tiles + row sums ----
    m_sb = [singles.tile([128, N], MM_DT, name=f"m{j}", tag=f"m{j}") for j in range(NT)]
    for j in range(NT):
        for q in range(4):
            pa = ps_z.tile([128, DO], FP32, name="pa", tag="pz")[:, 0:512]
            jsl = slice(j * 128, (j + 1) * 128)
            qsl = slice(q * 512, (q + 1) * 512)
            nc.tensor.matmul(pa, lhsT=ethi[:, jsl], rhs=ethi[:, qsl],
                             start=True, stop=False)
            nc.tensor.matmul(pa, lhsT=ethi[:, jsl], rhs=etlo[:, qsl],
                             start=False, stop=False)
            nc.tensor.matmul(pa, lhsT=etlo[:, jsl], rhs=ethi[:, qsl],
                             start=False, stop=True)
            dst = m_sb[j][:, q * 512:(q + 1) * 512]
            # exp(relu(a)) == max(exp(a), 1); row-sum accumulated in the max op
            nc.scalar.activation(out=dst, in_=pa,
                                 func=mybir.ActivationFunctionType.Exp)
            nc.vector.tensor_scalar(out=dst, in0=dst, scalar1=1.0, scalar2=None,
                                    op0=mybir.AluOpType.max,
                                    op1=mybir.AluOpType.add,
                                    accum_out=rsum4[:, j, q:q + 1])
    rsum = singles.tile([128, NT], FP32, tag="rsum")
    rneg2 = singles.tile([128, NT], FP32, tag="rneg2")
    nc.vector.reduce_sum(out=rsum, in_=rsum4, axis=mybir.AxisListType.X)
    nc.vector.reciprocal(out=rinv, in_=rsum)
    nc.vector.tensor_scalar_mul(out=rinv2, in0=rinv, scalar1=2.0)
    nc.vector.tensor_scalar_mul(out=rneg2, in0=rsum, scalar1=-0.5)
    # ACT-written copies so ACT consumers don't need a DVE wait
    rinv_act = singles.tile([128, NT], FP32, tag="rinv_act")
    rinv2_act = singles.tile([128, NT], FP32, tag="rinv2_act")
    nc.scalar.copy(out=rinv_act, in_=rinv)
    nc.scalar.copy(out=rinv2_act, in_=rinv2)
    # diag(-r/2) tiles: fold the X2 subtraction into the stage-4 matmul
    dneg = [singles.tile([128, 128], MM_DT, name=f"dneg{j}", tag=f"dneg{j}")
            for j in range(NT)]
    for j in range(NT):
        nc.vector.tensor_scalar_mul(out=dneg[j], in0=ident,
                                    scalar1=rneg2[:, j:j + 1])

    # ---- stage 2: load X2 ----
    x2_sb = [singles.tile([128, F], MM_DT, name=f"x2_{a}", tag=f"x2{a}") for a in range(NT)]
    for a in range(NT):
        nc.gpsimd.dma_start(out=x2_sb[a], in_=x2d[a * 128:(a + 1) * 128, :])

    # ---- stage 3: x1 = diag(1/r) M X2 (+ x1 transposes interleaved) ----
    x1_sb = [singles.tile([128, F], MM_DT, name=f"x1_{j}", tag=f"x1{j}") for j in range(NT)]
    for j in range(NT):
        pm = ps_mm.tile([128, F], FP32, tag="pm")
        for a in range(NT):
            nc.tensor.matmul(pm, lhsT=m_sb[a][:, j * 128:(j + 1) * 128],
                             rhs=x2_sb[a], start=(a == 0), stop=(a == NT - 1))
        nc.scalar.activation(out=x1_sb[j], in_=pm,
                             func=mybir.ActivationFunctionType.Copy,
                             scale=rinv_act[:, j:j + 1])
        for b in range(BC):
            col = b * N + j * 128
            tp = ps_t.tile([128, 128], MM_DT, tag="tp")
            nc.tensor.transpose(tp[64:128, :], x1_sb[j][:, b * C:(b + 1) * C],
                                ident, tile_position=(0, 64))
            nc.scalar.copy(out=xgt01[64:128, col:col + 128], in_=tp[64:128, :])

    # ---- stages 4+5+6 interleaved per n-chunk ----
    x2o_sb = [singles.tile([128, F], MM_DT, name=f"x2o_{j}", tag=f"x2o{j}") for j in range(NT)]
    for j in range(NT):
        pm = ps_mm.tile([128, F], FP32, tag="pm")
        for a in range(NT):
            nc.tensor.matmul(pm, lhsT=m_sb[a][:, j * 128:(j + 1) * 128],
                             rhs=x1_sb[a], start=(a == 0), stop=False)
        # psum += diag(-r/2) @ X2[j]  => psum = M@x1 - (r/2) X2[j]
        nc.tensor.matmul(pm, lhsT=dneg[j], rhs=x2_sb[j], start=False, stop=True)
        # x2 = (2/r) * psum
        nc.scalar.activation(out=x2o_sb[j], in_=pm,
                             func=mybir.ActivationFunctionType.Copy,
                             scale=rinv2_act[:, j:j + 1])
        for b in range(BC):
            col = b * N + j * 128
            tp2 = ps_t.tile([128, 128], MM_DT, tag="tp")
            nc.tensor.transpose(tp2[0:64, :], x2o_sb[j][:, b * C:(b + 1) * C],
                                ident, tile_position=(0, 0))
            nc.scalar.copy(out=xgt2[0:64, col:col + 128], in_=tp2[0:64, :])
        # stage 6 for this n-chunk: Z matmul + epilogue d-contraction
        erep = erep_pool.tile([128, C, EMBED], MM_DT, tag="erep")
        src6 = en[j * 128:(j + 1) * 128, :]
        bc_ap = bass.AP(tensor=src6.tensor, offset=src6.offset,
                        ap=[src6.ap[0], [0, C], src6.ap[1]])
        nc.gpsimd.dma_start(out=erep, in_=bc_ap)
        for b in range(BC):
            col = b * N + j * 128
            pz = ps_z.tile([128, DO], FP32, tag="pz")
            for h in range(2):
                sl = slice(h * 512, (h + 1) * 512)
                nc.tensor.matmul(pz[:, sl], lhsT=xgt01[:, col:col + 128],
                                 rhs=w1_sb[:, sl], start=True, stop=False)
                nc.tensor.matmul(pz[:, sl], lhsT=xgt2[:, col:col + 128],
                                 rhs=w2_sb[:, sl], start=False, stop=True)
            zsb = zsb_pool.tile([128, C, EMBED], MM_DT, tag="zsb")
            nc.scalar.copy(out=zsb, in_=pz)       # PSUM -> SBUF, cast bf16
            nc.vector.tensor_mul(zsb, zsb, erep)  # in-place *E (bf16 2x)
            # tree-reduce over d (innermost, 16): 3 halvings + final fp32 add
            for hw_ in (8, 4, 2):
                nc.vector.tensor_add(zsb[:, :, 0:hw_], zsb[:, :, 0:hw_],
                                     zsb[:, :, hw_:2 * hw_])
            tmp = tmp_pool.tile([128, C], FP32, tag="tmp")
            nc.vector.tensor_add(tmp, zsb[:, :, 0], zsb[:, :, 1])
            nc.sync.dma_start(out=out[b, j * 128:(j + 1) * 128, :], in_=tmp)


_NC_CACHE = None


def kernel(x, node_embedding, weights_pool, bias_pool):
    global _NC_CACHE
    if _NC_CACHE is None:
        _NC_CACHE = build_nc()
    nc = _NC_CACHE

    x = np.asarray(x, dtype=np.float32)
    E = np.asarray(node_embedding, dtype=np.float32)
    Wp = np.asarray(weights_pool, dtype=np.float32)
    bp = np.asarray(bias_pool, dtype=np.float32)

    import ml_dtypes
    etf = np.ascontiguousarray(E.T)
    eth = etf.astype(ml_dtypes.bfloat16).astype(np.float32)
    et = eth
    etlo = np.ascontiguousarray(etf - eth)
    # wpf[(k,i), (o,d)] = Wp[d,k,i,o]
    wpf = np.ascontiguousarray(Wp.transpose(1, 2, 3, 0).reshape(CHEB_K * C, DO))
    wpf1 = np.ascontiguousarray(wpf[0:128])
    # last row of wpf2 = bias pool flattened (o,d) to ride the ones-row
    bprow = np.ascontiguousarray(bp.T.reshape(1, DO))
    wpf2 = np.ascontiguousarray(np.concatenate([wpf[128:192], bprow], axis=0))

    in_maps = []
    for c in range(NCORES):
        xc = x[BC * c:BC * (c + 1)]
        in_maps.append({
            "x2d": np.ascontiguousarray(xc.transpose(1, 0, 2).reshape(N, F)),
            "xt": np.ascontiguousarray(xc.transpose(2, 0, 1)),
            "et": et, "etlo": etlo, "en": E, "wpf1": wpf1, "wpf2": wpf2,
        })
    res = run_bass_kernel_spmd(nc, in_maps, list(range(NCORES)))
    return np.concatenate([res.results[c]["out"] for c in range(NCORES)], axis=0)


if __name__ == "__main__":
    rng = np.random.default_rng(0)
    inputs = {
        "x": rng.standard_normal((B, N, C), dtype=np.float32),
        "node_embedding": rng.standard_normal((N, EMBED), dtype=np.float32),
        "weights_pool": (rng.standard_normal((EMBED, CHEB_K, C, C), dtype=np.float32) * 0.1),
        "bias_pool": (rng.standard_normal((EMBED, C), dtype=np.float32) * 0.1),
    }
    got = kernel(**inputs)
    print("out", got.shape, got.dtype, np.abs(got).max())


# revision 36
# speedup vs baseline: 1.2549x; 1.0327x over previous
"""AVWGCN Trainium2 kernel: adaptive-adjacency Chebyshev GCN.

Math (per core, batch-sharded over B: 8 batches/core):
  A = relu(E @ E^T) (symmetric), M = exp(A), r = rowsum(M), S = diag(1/r) M
  X2[m,(b,c)] = x[b,m,c]
  x1 = diag(1/r) (M @ X2)
  x2 = 2 diag(1/r) (M @ x1) - X2
  out[b,n,o] = sum_d E[n,d] * ( sum_{k,i} xg_k[n,(b,i)] Wp[d,k,i,o] + bp[d,o] )
Key tricks:
  - M symmetric -> its [n-part, m-free] tiles serve directly as matmul lhsT.
  - 1/r scalings are per-output-row -> per-partition DVE scalars.
  - gconv via Z-form: Z[bn,(o,d)] = XGT.T @ Wp_flat; bias folded in as a
    ones-row appended to the k2 K-chunk; epilogue contracts d with
    broadcast-replicated E on DVE.
  - x_g transposed to [ki-part, (b,m)-free] via PE transposes (k=0 slice comes
    pre-transposed from the host).
"""

from contextlib import ExitStack

import numpy as np

import concourse.bass as bass
import concourse.mybir as mybir
import concourse.tile as tile
from concourse.bass_utils import run_bass_kernel_spmd
from concourse.masks import make_identity

B, N, C, CHEB_K, EMBED = 64, 2048, 64, 3, 16
NCORES = 8
BC = B // NCORES            # batches per core
F = BC * C                  # 512: free width of X2 [m, (b,c)]
NT = N // 128               # 16 n-chunks
FP32 = mybir.dt.float32
BF16 = mybir.dt.bfloat16
MM_DT = BF16                # dtype of big-matmul operands
DO = C * EMBED              # 1024, Z free width, (o, d) ordered


_WAIT_CAP = {"InstDMACopy": 1}
_WAIT_SAFE = {"InstEventSemaphore", "InstCall",
              "InstUnconditionalBranch", "InstISA", "InstRegisterMove"}


def _split_excess_waits(nc):
    """Walrus rejects compute instructions carrying more sync waits than the
    ISA struct can encode. Hoist excess waits onto an inserted same-engine
    Drain immediately before the instruction (semantically identical)."""
    SyncInfo = None
    n_fix = 0
    for f in nc.m.functions:
        for blk in f.blocks:
            out_insts = []
            for inst in blk.instructions:
                tn = type(inst).__name__
                si = inst.sync_info
                w = list(si.on_wait) if (si is not None and si.on_wait) else []
                cap = _WAIT_CAP.get(tn, 1)
                if tn not in _WAIT_SAFE and len(w) > cap:
                    if SyncInfo is None:
                        SyncInfo = type(si)
                    for wx in w:
                        d = mybir.InstDrain(name=f"I-wsplit{nc.next_id()}",
                                            ins=[], outs=[])
                        d.engine = inst.engine
                        d.sync_info = SyncInfo(on_wait=[wx], on_update=[])
                        out_insts.append(d)
                    si.on_wait = []
                    n_fix += 1
                out_insts.append(inst)
            blk.instructions[:] = out_insts
    return n_fix


def build_nc():
    nc = bass.Bass()
    x2d = nc.dram_tensor("x2d", [N, F], FP32, kind="ExternalInput").ap()
    xt = nc.dram_tensor("xt", [C, BC, N], FP32, kind="ExternalInput").ap()
    et = nc.dram_tensor("et", [3 * EMBED, N], FP32, kind="ExternalInput").ap()
    etlo_d = nc.dram_tensor("etlo", [3 * EMBED, N], FP32, kind="ExternalInput").ap()
    en = nc.dram_tensor("en", [N, EMBED], FP32, kind="ExternalInput").ap()
    wpf1 = nc.dram_tensor("wpf1", [128, DO], FP32, kind="ExternalInput").ap()
    wpf2 = nc.dram_tensor("wpf2", [65, DO], FP32, kind="ExternalInput").ap()
    out = nc.dram_tensor("out", [BC, N, C], FP32, kind="ExternalOutput").ap()

    with tile.TileContext(nc) as tc:
        with ExitStack() as ctx:
            kernel_body(ctx, tc, out, x2d, xt, et, etlo_d, en, wpf1, wpf2)
    _split_excess_waits(nc)
    return nc


def kernel_body(ctx, tc, out, x2d, xt, et, etlo_d, en, wpf1, wpf2):
    nc = tc.nc

    singles = ctx.enter_context(tc.tile_pool(name="singles", bufs=1))
    erep_pool = ctx.enter_context(tc.tile_pool(name="erep", bufs=2))
    tmp_pool = ctx.enter_context(tc.tile_pool(name="tmp", bufs=8))
    zsb_pool = ctx.enter_context(tc.tile_pool(name="zsb", bufs=3))
    ps_mm = ctx.enter_context(tc.tile_pool(name="ps_mm", bufs=2, space="PSUM"))
    ps_t = ctx.enter_context(tc.tile_pool(name="ps_t", bufs=2, space="PSUM"))
    ps_z = ctx.enter_context(tc.tile_pool(name="ps_z", bufs=2, space="PSUM"))

    # ---- constants ----
    ident = singles.tile([128, 128], MM_DT, tag="ident")
    make_identity(nc, ident)
    # split-precision E^T, K-stacked: one K=48 matmul computes
    # EhiEhi^T + EhiElo^T + EloEhi^T (lhsT=[Ehi;Ehi;Elo], rhs=[Ehi;Elo;Ehi])
    ethi = singles.tile([3 * EMBED, N], MM_DT, tag="ethi")
    etlo = singles.tile([3 * EMBED, N], MM_DT, tag="etlo")
    nc.gpsimd.dma_start(out=ethi, in_=et)
    nc.gpsimd.dma_start(out=etlo, in_=etlo_d)
    w1_sb = singles.tile([128, DO], MM_DT, tag="w1")
    nc.gpsimd.dma_start(out=w1_sb, in_=wpf1)
    w2_sb = singles.tile([65, DO], MM_DT, tag="w2")
    nc.gpsimd.dma_start(out=w2_sb, in_=wpf2)

    # XGT tiles: k0+k1 interleaved on partitions, k2 (+ones row) separate
    xgt01 = singles.tile([128, BC * N], MM_DT, tag="xgt01")      # 4 MiB
    xgt2 = singles.tile([65, BC * N], MM_DT, tag="xgt2")         # ~2 MiB
    # k=0 slice comes pre-transposed from host; cast to bf16 during DMA
    nc.gpsimd.dma_start(out=xgt01[0:C, :], in_=xt.rearrange("c b n -> c (b n)"))
    nc.vector.memset(xgt2[64:65, :], 1.0)                        # bias ones-row

    rsum4 = singles.tile([128, NT, 4], FP32, tag="rsum4")
    rinv = singles.tile([128, NT], FP32, tag="rinv")
    rinv2 = singles.tile([128, NT], FP32, tag="rinv2")

    # ---- stage 1: M = exp(relu(E E^T)) as bf16 tiles + row sums ----
    m_sb = [singles.tile([128, N], MM_DT, name=f"m{j}", tag=f"m{j}") for j in range(NT)]
    for j in range(NT):
        for q in range(4):
            pa = ps_z.tile([128, DO], FP32, name="pa", tag="pz")[:, 0:512]
            jsl = slice(j * 128, (j + 1) * 128)
            qsl = slice(q * 512, (q + 1) * 512)
            nc.tensor.matmul(pa, lhsT=ethi[:, jsl], rhs=etlo[:, qsl],
                             start=True, stop=True)
            dst = m_sb[j][:, q * 512:(q + 1) * 512]
            # exp(relu(a)) == max(exp(a), 1); row-sum accumulated in the max op
            nc.scalar.activation(out=dst, in_=pa,
                                 func=mybir.ActivationFunctionType.Exp)
            nc.vector.tensor_scalar(out=dst, in0=dst, scalar1=1.0, scalar2=None,
                                    op0=mybir.AluOpType.max,
                                    op1=mybir.AluOpType.add,
                                    accum_out=rsum4[:, j, q:q + 1])
    rsum = singles.tile([128, NT], FP32, tag="rsum")
    rneg2 = singles.tile([128, NT], FP32, tag="rneg2")
    nc.vector.reduce_sum(out=rsum, in_=rsum4, axis=mybir.AxisListType.X)
    nc.vector.reciprocal(out=rinv, in_=rsum)
    nc.vector.tensor_scalar_mul(out=rinv2, in0=rinv, scalar1=2.0)
    nc.vector.tensor_scalar_mul(out=rneg2, in0=rsum, scalar1=-0.5)
    # ACT-written copies so ACT consumers don't need a DVE wait
    rinv_act = singles.tile([128, NT], FP32, tag="rinv_act")
    rinv2_act = singles.tile([128, NT], FP32, tag="rinv2_act")
    nc.scalar.copy(out=rinv_act, in_=rinv)
    nc.scalar.copy(out=rinv2_act, in_=rinv2)
    # diag(-r/2) tiles: fold the X2 subtraction into the stage-4 matmul
    dneg = [singles.tile([128, 128], MM_DT, name=f"dneg{j}", tag=f"dneg{j}")
            for j in range(NT)]
    for j in range(NT):
        nc.vector.tensor_scalar_mul(out=dneg[j], in0=ident,
                                    scalar1=rneg2[:, j:j + 1])

    # ---- stage 2: load X2 ----
    x2_sb = [singles.tile([128, F], MM_DT, name=f"x2_{a}", tag=f"x2{a}") for a in range(NT)]
    for a in range(NT):
        nc.gpsimd.dma_start(out=x2_sb[a], in_=x2d[a * 128:(a + 1) * 128, :])

    # ---- stage 3: x1 = diag(1/r) M X2 (+ x1 transposes interleaved) ----
    x1_sb = [singles.tile([128, F], MM_DT, name=f"x1_{j}", tag=f"x1{j}") for j in range(NT)]
    for j in range(NT):
        pm = ps_mm.tile([128, F], FP32, tag="pm")
        for a in range(NT):
            nc.tensor.matmul(pm, lhsT=m_sb[a][:, j * 128:(j + 1) * 128],
                             rhs=x2_sb[a], start=(a == 0), stop=(a == NT - 1))
        nc.scalar.activation(out=x1_sb[j], in_=pm,
                             func=mybir.ActivationFunctionType.Copy,
                             scale=rinv_act[:, j:j + 1])
        for b in range(BC):
            col = b * N + j * 128
            tp = ps_t.tile([128, 128], MM_DT, tag="tp")
            nc.tensor.transpose(tp[64:128, :], x1_sb[j][:, b * C:(b + 1) * C],
                                ident, tile_position=(0, 64))
            nc.scalar.copy(out=xgt01[64:128, col:col + 128], in_=tp[64:128, :])

    # ---- stages 4+5+6 interleaved per n-chunk ----
    x2o_sb = [singles.tile([128, F], MM_DT, name=f"x2o_{j}", tag=f"x2o{j}") for j in range(NT)]
    for j in range(NT):
        pm = ps_mm.tile([128, F], FP32, tag="pm")
        for a in range(NT):
            nc.tensor.matmul(pm, lhsT=m_sb[a][:, j * 128:(j + 1) * 128],
                             rhs=x1_sb[a], start=(a == 0), stop=False)
        # psum += diag(-r/2) @ X2[j]  => psum = M@x1 - (r/2) X2[j]
        nc.tensor.matmul(pm, lhsT=dneg[j], rhs=x2_sb[j], start=False, stop=True)
        # x2 = (2/r) * psum
        nc.scalar.activation(out=x2o_sb[j], in_=pm,
                             func=mybir.ActivationFunctionType.Copy,
                             scale=rinv2_act[:, j:j + 1])
        for b in range(BC):
            col = b * N + j * 128
            tp2 = ps_t.tile([128, 128], MM_DT, tag="tp")
            nc.tensor.transpose(tp2[0:64, :], x2o_sb[j][:, b * C:(b + 1) * C],
                                ident, tile_position=(0, 0))
            nc.scalar.copy(out=xgt2[0:64, col:col + 128], in_=tp2[0:64, :])
        # stage 6 for this n-chunk: Z matmul + epilogue d-contraction
        erep = erep_pool.tile([128, C, EMBED], MM_DT, tag="erep")
        src6 = en[j * 128:(j + 1) * 128, :]
        bc_ap = bass.AP(tensor=src6.tensor, offset=src6.offset,
                        ap=[src6.ap[0], [0, C], src6.ap[1]])
        nc.gpsimd.dma_start(out=erep, in_=bc_ap)
        for b in range(BC):
            col = b * N + j * 128
            pz = ps_z.tile([128, DO], FP32, tag="pz")
            for h in range(2):
                sl = slice(h * 512, (h + 1) * 512)
                nc.tensor.matmul(pz[:, sl], lhsT=xgt01[:, col:col + 128],
                                 rhs=w1_sb[:, sl], start=True, stop=False)
                nc.tensor.matmul(pz[:, sl], lhsT=xgt2[:, col:col + 128],
                                 rhs=w2_sb[:, sl], start=False, stop=True)
            zsb = zsb_pool.tile([128, C, EMBED], MM_DT, tag="zsb")
            nc.scalar.copy(out=zsb, in_=pz)       # PSUM -> SBUF, cast bf16
            nc.vector.tensor_mul(zsb, zsb, erep)  # in-place *E (bf16 2x)
            # tree-reduce over d (innermost, 16): 3 halvings + final fp32 add
            for hw_ in (8, 4, 2):
                nc.vector.tensor_add(zsb[:, :, 0:hw_], zsb[:, :, 0:hw_],
                                     zsb[:, :, hw_:2 * hw_])
            tmp = tmp_pool.tile([128, C], FP32, tag="tmp")
            nc.vector.tensor_add(tmp, zsb[:, :, 0], zsb[:, :, 1])
            nc.sync.dma_start(out=out[b, j * 128:(j + 1) * 128, :], in_=tmp)


_NC_CACHE = None


def kernel(x, node_embedding, weights_pool, bias_pool):
    global _NC_CACHE
    if _NC_CACHE is None:
        _NC_CACHE = build_nc()
    nc = _NC_CACHE

    x = np.asarray(x, dtype=np.float32)
    E = np.asarray(node_embedding, dtype=np.float32)
    Wp = np.asarray(weights_pool, dtype=np.float32)
    bp = np.asarray(bias_pool, dtype=np.float32)

    import ml_dtypes
    etf = np.ascontiguousarray(E.T)
    eth = etf.astype(ml_dtypes.bfloat16).astype(np.float32)
    elo = (etf - eth).astype(np.float32)
    et = np.ascontiguousarray(np.concatenate([eth, eth, elo], axis=0))
    etlo = np.ascontiguousarray(np.concatenate([eth, elo, eth], axis=0))
    # wpf[(k,i), (o,d)] = Wp[d,k,i,o]
    wpf = np.ascontiguousarray(Wp.transpose(1, 2, 3, 0).reshape(CHEB_K * C, DO))
    wpf1 = np.ascontiguousarray(wpf[0:128])
    # last row of wpf2 = bias pool flattened (o,d) to ride the ones-row
    bprow = np.ascontiguousarray(bp.T.reshape(1, DO))
    wpf2 = np.ascontiguousarray(np.concatenate([wpf[128:192], bprow], axis=0))

    in_maps = []
    for c in range(NCORES):
        xc = x[BC * c:BC * (c + 1)]
        in_maps.append({
            "x2d": np.ascontiguousarray(xc.transpose(1, 0, 2).reshape(N, F)),
            "xt": np.ascontiguousarray(xc.transpose(2, 0, 1)),
            "et": et, "etlo": etlo, "en": E, "wpf1": wpf1, "wpf2": wpf2,
        })
    res = run_bass_kernel_spmd(nc, in_maps, list(range(NCORES)))
    return np.concatenate([res.results[c]["out"] for c in range(NCORES)], axis=0)


if __name__ == "__main__":
    rng = np.random.default_rng(0)
    inputs = {
        "x": rng.standard_normal((B, N, C), dtype=np.float32),
        "node_embedding": rng.standard_normal((N, EMBED), dtype=np.float32),
        "weights_pool": (rng.standard_normal((EMBED, CHEB_K, C, C), dtype=np.float32) * 0.1),
        "bias_pool": (rng.standard_normal((EMBED, C), dtype=np.float32) * 0.1),
    }
    got = kernel(**inputs)
    print("out", got.shape, got.dtype, np.abs(got).max())
